# revision 17
# baseline (speedup 1.0000x reference)
"""Single-head causal attention on 8 TRN2 NeuronCores.

Problem: x[8,2048,1024] @ Wq/Wk/Wv[1024,64] -> causal softmax attention -> out[8,2048,64].
Sharding: data-parallel over batch B=8, one batch element per core; weights replicated.

Per-core design (T=2048, C=1024, H=64), v4 tuned for dense PE occupancy:
 - x loaded f32 per 128-row block, alternating sync-HWDGE / gpsimd-SWDGE queues
   (parallel descriptor gen), cast f32->bf16 on DVE, transposed on PE into xT.
 - transposes issued tt-major so each arriving block unlocks 8 transposes;
   4 PSUM banks hold the 4 jc-pair staging tiles ([128,1024] bf16 = 1 bank),
   each drained by one wide DVE copy.
 - PE warmup: dummy identity transposes at start flip the HAM clock gate to
   2.4GHz before real work arrives.
 - q,k projected together (stationary [Wq|Wk]) then split into per-chunk qT/kT
   tiles (no cross-chunk WAR). PSUM->SBUF copies ride ScalarE early (idle) and
   DVE late (ScalarE exp-bound).
 - scores TRANSPOSED: weiT[s,t] = kT.T@qT per (s-block, t-chunk), bf16 PSUM
   output (scores don't accumulate; halves PSUM footprint); two s-blocks per
   [128,1024] tile so each ScalarE exp covers 1024 cols. exp folds in the
   C**-0.5 scale; no max subtraction (scores O(1), softmax shift-invariant).
 - causal mask: fully-masked blocks skipped, PV streams only [lo:] of diagonal
   tiles, below-diagonal of the 128x128 diagonal zeroed by GpSimd affine_select;
   softmax denominator via an extra ones column on the PV stationary [v | 1].
 - final normalization (divide by sums + transpose [65,512]) on host.
"""

import numpy as np

import concourse.bass as bass
import concourse.mybir as mybir
import concourse.tile as tile
from concourse import bacc
from concourse.masks import make_identity
from contextlib import ExitStack

P = 128
T = 2048
C = 1024
H = 64
B = 8
NC = C // P          # 8 c-tiles
NT = T // P          # 16 s/t 128-blocks
CH = 512             # t-chunk width
NCH = T // CH        # 4 chunks
BPC = CH // P        # 4 blocks per chunk
SCALE = float(C) ** -0.5
F32 = mybir.dt.float32
BF16 = mybir.dt.bfloat16
EXP = mybir.ActivationFunctionType.Exp
N_WARM = 16          # PE warmup transposes


def build_nc():
    nc = bacc.Bacc(None, target_bir_lowering=False)
    x = nc.dram_tensor("x", [T, C], F32, kind="ExternalInput")
    wq_d = nc.dram_tensor("Wq", [C, H], F32, kind="ExternalInput")
    wk_d = nc.dram_tensor("Wk", [C, H], F32, kind="ExternalInput")
    wv_d = nc.dram_tensor("Wv", [C, H], F32, kind="ExternalInput")
    out_d = nc.dram_tensor("outT", [H + 1, T], F32, kind="ExternalOutput")

    with tile.TileContext(nc) as tc, ExitStack() as ctx:
        consts = ctx.enter_context(tc.tile_pool(name="consts", bufs=1))
        xbp = ctx.enter_context(tc.tile_pool(name="xbp", bufs=16))
        xcp = ctx.enter_context(tc.tile_pool(name="xcp", bufs=16))
        xtp = ctx.enter_context(tc.tile_pool(name="xtp", bufs=8))
        persist = ctx.enter_context(tc.tile_pool(name="persist", bufs=1))
        wei = ctx.enter_context(tc.tile_pool(name="wei", bufs=8))
        vtsp = ctx.enter_context(tc.tile_pool(name="vtsp", bufs=2))
        fin = ctx.enter_context(tc.tile_pool(name="fin", bufs=2))
        # PSUM: 8 banks total; ptx 4 + ppj 1 + psc 2 + pout 1 = 8.
        ptx = ctx.enter_context(tc.tile_pool(name="ptx", bufs=4, space="PSUM"))
        ppj = ctx.enter_context(tc.tile_pool(name="ppj", bufs=1, space="PSUM"))
        psc = ctx.enter_context(tc.tile_pool(name="psc", bufs=2, space="PSUM"))
        pout = ctx.enter_context(tc.tile_pool(name="pout", bufs=1, space="PSUM"))

        # ---- x block loads: first chunk ahead of weights (PE starts sooner);
        # even blocks on the sync HWDGE ring, odd on gpsimd SWDGE (parallel gen)
        def load_block(blk):
            t_ = xbp.tile([P, C], F32, tag="xb", name=f"xb{blk}")
            eng = nc.sync if (blk < BPC or blk % 2 == 0) else nc.gpsimd
            eng.dma_start(out=t_, in_=x[blk * P : (blk + 1) * P, :])
            return t_

        xb = [None] * NT
        for blk in range(BPC):
            xb[blk] = load_block(blk)

        # ---- constants
        ident_f = consts.tile([P, P], F32)
        make_identity(nc, ident_f)
        ident_b = consts.tile([P, P], BF16)
        nc.vector.tensor_copy(out=ident_b, in_=ident_f)

        # weights: gpsimd SWDGE (the strided 256B-descriptor pattern would
        # block the HWDGE ring FIFO and stall the x block loads behind it)
        wq_f = consts.tile([P, NC, H], F32)
        wk_f = consts.tile([P, NC, H], F32)
        wv_f = consts.tile([P, NC, H], F32)
        nc.gpsimd.dma_start(out=wq_f, in_=wq_d.rearrange("(j p) h -> p j h", p=P))
        nc.gpsimd.dma_start(out=wk_f, in_=wk_d.rearrange("(j p) h -> p j h", p=P))
        nc.gpsimd.dma_start(out=wv_f, in_=wv_d.rearrange("(j p) h -> p j h", p=P))

        # remaining x blocks
        for blk in range(BPC, NT):
            xb[blk] = load_block(blk)

        # ---- PE warmup: dummy transposes to flip the HAM clock gate early
        for _ in range(N_WARM):
            wt = ptx.tile([P, 2 * CH], BF16, tag="tr")
            nc.tensor.transpose(wt[:, 0:P], ident_b, ident_b)

        # ---- f32 -> bf16 casts on DVE (2x mode); chunk 0 upfront, the rest
        # issued just-in-time inside the previous chunk's body so urgent DVE
        # copies are not stuck behind far-future casts in the engine FIFO
        xc = [None] * NT

        def cast_block(blk):
            xc[blk] = xcp.tile([P, C], BF16, tag="xc", name=f"xc{blk}")
            nc.vector.tensor_copy(out=xc[blk], in_=xb[blk])

        for blk in range(BPC):
            cast_block(blk)

        # weight casts after the chunk-0 x casts (DVE FIFO order matters)
        wqk_sb = consts.tile([P, NC, P], BF16)
        nc.vector.tensor_copy(out=wqk_sb[:, :, 0:H], in_=wq_f)
        nc.vector.tensor_copy(out=wqk_sb[:, :, H : 2 * H], in_=wk_f)
        wv_sb = consts.tile([P, NC, H], BF16)
        nc.vector.tensor_copy(out=wv_sb, in_=wv_f)

        # per-chunk persistent projections (separate tiles -> no cross-chunk WAR)
        qT_c = [persist.tile([H, CH], BF16, tag=f"qT{tb}", name=f"qT{tb}") for tb in range(NCH)]
        kT_c = [persist.tile([H, CH], BF16, tag=f"kT{tb}", name=f"kT{tb}") for tb in range(NCH)]
        v_c = [persist.tile([P, BPC, H + 1], BF16, tag=f"v{tb}", name=f"v{tb}") for tb in range(NCH)]
        for tb in range(NCH):
            nc.gpsimd.memset(v_c[tb][:, :, H : H + 1], 1.0)  # denominator column

        for tb in range(NCH):
            blk0 = tb * BPC
            # PSUM->SBUF copies: ScalarE while it is idle (early chunks), DVE
            # once ScalarE is exp-bound (late chunks)
            pj_copy = nc.scalar.copy if tb < 2 else nc.vector.tensor_copy
            # ---- transpose x chunk into xT c-tiles; tt-major so each arriving
            # block unlocks 8 transposes; jc-pairs staged per PSUM bank
            pts = [ptx.tile([P, 2 * CH], BF16, tag="tr", name=f"pt{tb}_{jp}")
                   for jp in range(NC // 2)]
            for tt in range(BPC):
                for jc in range(NC):
                    nc.tensor.transpose(
                        pts[jc // 2][:, (jc % 2) * CH + tt * P : (jc % 2) * CH + (tt + 1) * P],
                        xc[blk0 + tt][:, jc * P : (jc + 1) * P],
                        ident_b,
                    )
            xt = [None] * (NC // 2)
            for jp in range(NC // 2):
                xt[jp] = xtp.tile([P, 2 * CH], BF16, tag="xt", name=f"xt{jp}")
                nc.vector.tensor_copy(out=xt[jp], in_=pts[jp])
            # ---- q|k projection: stationary [Wq|Wk] per c-tile, stream xT
            pqk = ppj.tile([P, CH], F32, tag="pj")
            for jc in range(NC):
                nc.tensor.matmul(pqk, lhsT=wqk_sb[:, jc, :],
                                 rhs=xt[jc // 2][:, (jc % 2) * CH : (jc % 2 + 1) * CH],
                                 start=(jc == 0), stop=(jc == NC - 1))
            pj_copy(out=qT_c[tb], in_=pqk[0:H, :])
            pj_copy(out=kT_c[tb], in_=pqk[H : 2 * H, :])
            # prefetch-cast the next chunk's blocks (after this chunk's copies)
            if tb + 1 < NCH:
                for tt in range(BPC):
                    cast_block((tb + 1) * BPC + tt)
            # ---- v projection, then small transposes to v natural [s, 64]
            pv = ppj.tile([P, CH], F32, tag="pj")
            for jc in range(NC):
                nc.tensor.matmul(pv[0:H, :], lhsT=wv_sb[:, jc, :],
                                 rhs=xt[jc // 2][:, (jc % 2) * CH : (jc % 2 + 1) * CH],
                                 start=(jc == 0), stop=(jc == NC - 1))
            vts = vtsp.tile([H, CH], BF16, tag="vt")
            pj_copy(out=vts, in_=pv[0:H, :])
            pvn = ptx.tile([P, 2 * CH], BF16, tag="tr")
            for tt in range(BPC):
                nc.tensor.transpose(pvn[:, tt * H : (tt + 1) * H],
                                    vts[:, tt * P : (tt + 1) * P],
                                    ident_b[0:H, 0:H])
            nc.vector.tensor_copy(out=v_c[tb][:, :, 0:H], in_=pvn[:, 0 : BPC * H])
            # ---- scores (transposed) + exp + PV, per s-block
            po = pout.tile([H + 1, CH], F32, tag="po")
            nsb = (tb + 1) * BPC
            for si in range(nsb):
                lo = max(0, (si - tb * BPC) * P)
                ps = psc.tile([P, CH], F32, tag="sc")
                w = wei.tile([P, CH], BF16, tag="w")
                nc.tensor.matmul(
                    ps[:, lo:CH],
                    lhsT=kT_c[si // BPC][:, (si % BPC) * P : (si % BPC + 1) * P],
                    rhs=qT_c[tb][:, lo:CH],
                    start=True, stop=True,
                )
                nc.scalar.activation(out=w[:, lo:CH], in_=ps[:, lo:CH],
                                     func=EXP, scale=SCALE)
                if si >= tb * BPC:  # diagonal block: zero below-diagonal (t < s)
                    nc.gpsimd.affine_select(
                        out=w[:, lo : lo + P],
                        in_=w[:, lo : lo + P],
                        compare_op=mybir.AluOpType.is_ge,
                        fill=0.0,
                        base=0,
                        # keep where (col - row) >= 0
                        pattern=[[1, P]],
                        channel_multiplier=-1,
                    )
                nc.tensor.matmul(po[:, lo:CH], lhsT=v_c[si // BPC][:, si % BPC, :],
                                 rhs=w[:, lo:CH],
                                 start=(si == 0), stop=(si == nsb - 1))
            # ---- finalize chunk: copy outT+sums to SBUF and store; the cheap
            # per-row divide + transpose happens host-side during unshard.
            os_ = fin.tile([H + 1, CH], F32, tag="ot")
            nc.vector.tensor_copy(out=os_, in_=po)
            nc.sync.dma_start(out=out_d[:, tb * CH : (tb + 1) * CH], in_=os_)
    return nc


_NC_CACHE = []


def _get_nc():
    if not _NC_CACHE:
        nc = build_nc()
        nc.finalize()  # bacc compile: register allocation, DCE
        _NC_CACHE.append(nc)
    return _NC_CACHE[0]


def kernel(**inputs):
    x = np.ascontiguousarray(np.asarray(inputs["x"], dtype=np.float32))
    wq = np.ascontiguousarray(np.asarray(inputs["Wq"], dtype=np.float32))
    wk = np.ascontiguousarray(np.asarray(inputs["Wk"], dtype=np.float32))
    wv = np.ascontiguousarray(np.asarray(inputs["Wv"], dtype=np.float32))
    from concourse.bass_utils import run_bass_kernel_spmd

    nc = _get_nc()
    in_maps = [{"x": np.ascontiguousarray(x[b]), "Wq": wq, "Wk": wk, "Wv": wv} for b in range(B)]
    res = run_bass_kernel_spmd(nc, in_maps, core_ids=list(range(B)))
    return postprocess([res.results[b]["outT"] for b in range(B)])


def postprocess(outTs):
    outs = []
    for oT in outTs:
        outs.append((oT[0:H, :] / oT[H : H + 1, :]).T.astype(np.float32))
    return np.stack(outs, axis=0)


if __name__ == "__main__":
    import os
    os.makedirs("/tmp/neffdir3", exist_ok=True)
    from concourse.bass_utils import compile_bass_kernel

    nc = _get_nc()
    print("build OK, instructions:",
          sum(len(bb.instructions) for bb in nc.m.functions[0].blocks))
    print("COMPILED:", compile_bass_kernel(nc, "/tmp/neffdir3"))


# revision 22
# speedup vs baseline: 1.2434x; 1.2434x over previous
"""Single-head causal attention on 8 TRN2 NeuronCores.

Problem: x[8,2048,1024] @ Wq/Wk/Wv[1024,64] -> causal softmax attention -> out[8,2048,64].
Sharding: data-parallel over batch B=8, one batch element per core; weights replicated.

Per-core design (T=2048, C=1024, H=64), v4 tuned for dense PE occupancy:
 - x loaded f32 per 128-row block, alternating sync-HWDGE / gpsimd-SWDGE queues
   (parallel descriptor gen), cast f32->bf16 on DVE, transposed on PE into xT.
 - transposes issued tt-major so each arriving block unlocks 8 transposes;
   4 PSUM banks hold the 4 jc-pair staging tiles ([128,1024] bf16 = 1 bank),
   each drained by one wide DVE copy.
 - PE warmup: dummy identity transposes at start flip the HAM clock gate to
   2.4GHz before real work arrives.
 - q,k projected together (stationary [Wq|Wk]) then split into per-chunk qT/kT
   tiles (no cross-chunk WAR). PSUM->SBUF copies ride ScalarE early (idle) and
   DVE late (ScalarE exp-bound).
 - scores TRANSPOSED: weiT[s,t] = kT.T@qT per (s-block, t-chunk), bf16 PSUM
   output (scores don't accumulate; halves PSUM footprint); two s-blocks per
   [128,1024] tile so each ScalarE exp covers 1024 cols. exp folds in the
   C**-0.5 scale; no max subtraction (scores O(1), softmax shift-invariant).
 - causal mask: fully-masked blocks skipped, PV streams only [lo:] of diagonal
   tiles, below-diagonal of the 128x128 diagonal zeroed by GpSimd affine_select;
   softmax denominator via an extra ones column on the PV stationary [v | 1].
 - final normalization (divide by sums + transpose [65,512]) on host.
"""

import numpy as np

import concourse.bass as bass
import concourse.mybir as mybir
import concourse.tile as tile
from concourse import bacc
from concourse.masks import make_identity
from contextlib import ExitStack

P = 128
T = 2048
C = 1024
H = 64
B = 8
NC = C // P          # 8 c-tiles
NT = T // P          # 16 s/t 128-blocks
CH = 512             # t-chunk width
NCH = T // CH        # 4 chunks
BPC = CH // P        # 4 blocks per chunk
SCALE = float(C) ** -0.5
F32 = mybir.dt.float32
BF16 = mybir.dt.bfloat16
EXP = mybir.ActivationFunctionType.Exp
N_WARM = 16          # PE warmup transposes


def build_nc():
    nc = bacc.Bacc(None, target_bir_lowering=False)
    x = nc.dram_tensor("x", [T, C], F32, kind="ExternalInput")
    # weights pre-packed on host into the on-chip layout (contiguous DMA)
    wqk_d = nc.dram_tensor("Wqk", [P, NC, 2 * H], F32, kind="ExternalInput")
    wv_d = nc.dram_tensor("Wv2", [P, NC, H], F32, kind="ExternalInput")
    out_d = nc.dram_tensor("outT", [H + 1, T], F32, kind="ExternalOutput")

    with tile.TileContext(nc) as tc, ExitStack() as ctx:
        consts = ctx.enter_context(tc.tile_pool(name="consts", bufs=1))
        xbp = ctx.enter_context(tc.tile_pool(name="xbp", bufs=16))
        xcp = ctx.enter_context(tc.tile_pool(name="xcp", bufs=16))
        xtp = ctx.enter_context(tc.tile_pool(name="xtp", bufs=8))
        persist = ctx.enter_context(tc.tile_pool(name="persist", bufs=1))
        wei = ctx.enter_context(tc.tile_pool(name="wei", bufs=8))
        vtsp = ctx.enter_context(tc.tile_pool(name="vtsp", bufs=2))
        fin = ctx.enter_context(tc.tile_pool(name="fin", bufs=2))
        # PSUM: 8 banks total; ptx 4 + ppj 1 + psc 2 + pout 1 = 8.
        ptx = ctx.enter_context(tc.tile_pool(name="ptx", bufs=4, space="PSUM"))
        ppj = ctx.enter_context(tc.tile_pool(name="ppj", bufs=1, space="PSUM"))
        psc = ctx.enter_context(tc.tile_pool(name="psc", bufs=2, space="PSUM"))
        pout = ctx.enter_context(tc.tile_pool(name="pout", bufs=1, space="PSUM"))

        # ---- x block loads: first chunk ahead of weights (PE starts sooner);
        # even blocks on the sync HWDGE ring, odd on gpsimd SWDGE (parallel gen)
        def load_block(blk):
            t_ = xbp.tile([P, C], F32, tag="xb", name=f"xb{blk}")
            eng = nc.sync if (blk < BPC or blk % 2 == 0) else nc.gpsimd
            eng.dma_start(out=t_, in_=x[blk * P : (blk + 1) * P, :])
            return t_

        xb = [None] * NT
        for blk in range(BPC):
            xb[blk] = load_block(blk)

        # ---- constants
        ident_f = consts.tile([P, P], F32)
        make_identity(nc, ident_f)
        ident_b = consts.tile([P, P], BF16)
        nc.vector.tensor_copy(out=ident_b, in_=ident_f)

        # weights: host-packed layout, two small contiguous HWDGE loads
        wqk_f = consts.tile([P, NC, 2 * H], F32)
        wv_f = consts.tile([P, NC, H], F32)
        nc.sync.dma_start(out=wqk_f, in_=wqk_d[:, :, :])
        nc.sync.dma_start(out=wv_f, in_=wv_d[:, :, :])

        # remaining x blocks
        for blk in range(BPC, NT):
            xb[blk] = load_block(blk)

        # ---- PE warmup: dummy transposes to flip the HAM clock gate early
        for _ in range(N_WARM):
            wt = ptx.tile([P, 2 * CH], BF16, tag="tr")
            nc.tensor.transpose(wt[:, 0:P], ident_b, ident_b)

        # ---- f32 -> bf16 casts on DVE (2x mode); chunk 0 upfront, the rest
        # issued just-in-time inside the previous chunk's body so urgent DVE
        # copies are not stuck behind far-future casts in the engine FIFO
        xc = [None] * NT

        def cast_block(blk):
            xc[blk] = xcp.tile([P, C], BF16, tag="xc", name=f"xc{blk}")
            nc.vector.tensor_copy(out=xc[blk], in_=xb[blk])

        for blk in range(BPC):
            cast_block(blk)

        # weight casts after the chunk-0 x casts (DVE FIFO order matters)
        wqk_sb = consts.tile([P, NC, P], BF16)
        nc.vector.tensor_copy(out=wqk_sb, in_=wqk_f)
        wv_sb = consts.tile([P, NC, H], BF16)
        nc.vector.tensor_copy(out=wv_sb, in_=wv_f)

        # per-chunk persistent projections (separate tiles -> no cross-chunk WAR)
        qT_c = [persist.tile([H, CH], BF16, tag=f"qT{tb}", name=f"qT{tb}") for tb in range(NCH)]
        kT_c = [persist.tile([H, CH], BF16, tag=f"kT{tb}", name=f"kT{tb}") for tb in range(NCH)]
        v_c = [persist.tile([P, BPC, H + 1], BF16, tag=f"v{tb}", name=f"v{tb}") for tb in range(NCH)]
        for tb in range(NCH):
            nc.gpsimd.memset(v_c[tb][:, :, H : H + 1], 1.0)  # denominator column

        for tb in range(NCH):
            blk0 = tb * BPC
            # PSUM->SBUF copies: ScalarE while it is idle (early chunks), DVE
            # once ScalarE is exp-bound (late chunks)
            pj_copy = nc.scalar.copy if tb < 2 else nc.vector.tensor_copy
            # ---- transpose x chunk into xT c-tiles; tt-major so each arriving
            # block unlocks 8 transposes; jc-pairs staged per PSUM bank
            pts = [ptx.tile([P, 2 * CH], BF16, tag="tr", name=f"pt{tb}_{jp}")
                   for jp in range(NC // 2)]
            for tt in range(BPC):
                for jc in range(NC):
                    nc.tensor.transpose(
                        pts[jc // 2][:, (jc % 2) * CH + tt * P : (jc % 2) * CH + (tt + 1) * P],
                        xc[blk0 + tt][:, jc * P : (jc + 1) * P],
                        ident_b,
                    )
            xt = [None] * (NC // 2)
            for jp in range(NC // 2):
                xt[jp] = xtp.tile([P, 2 * CH], BF16, tag="xt", name=f"xt{jp}")
                nc.vector.tensor_copy(out=xt[jp], in_=pts[jp])
            # ---- q|k projection: stationary [Wq|Wk] per c-tile, stream xT
            pqk = ppj.tile([P, CH], F32, tag="pj")
            for jc in range(NC):
                nc.tensor.matmul(pqk, lhsT=wqk_sb[:, jc, :],
                                 rhs=xt[jc // 2][:, (jc % 2) * CH : (jc % 2 + 1) * CH],
                                 start=(jc == 0), stop=(jc == NC - 1))
            pj_copy(out=qT_c[tb], in_=pqk[0:H, :])
            pj_copy(out=kT_c[tb], in_=pqk[H : 2 * H, :])
            # prefetch-cast the next chunk's blocks (after this chunk's copies)
            if tb + 1 < NCH:
                for tt in range(BPC):
                    cast_block((tb + 1) * BPC + tt)
            # ---- v projection, then small transposes to v natural [s, 64]
            pv = ppj.tile([P, CH], F32, tag="pj")
            for jc in range(NC):
                nc.tensor.matmul(pv[0:H, :], lhsT=wv_sb[:, jc, :],
                                 rhs=xt[jc // 2][:, (jc % 2) * CH : (jc % 2 + 1) * CH],
                                 start=(jc == 0), stop=(jc == NC - 1))
            vts = vtsp.tile([H, CH], BF16, tag="vt")
            pj_copy(out=vts, in_=pv[0:H, :])
            pvn = ptx.tile([P, 2 * CH], BF16, tag="tr")
            for tt in range(BPC):
                nc.tensor.transpose(pvn[:, tt * H : (tt + 1) * H],
                                    vts[:, tt * P : (tt + 1) * P],
                                    ident_b[0:H, 0:H])
            nc.vector.tensor_copy(out=v_c[tb][:, :, 0:H], in_=pvn[:, 0 : BPC * H])
            # ---- scores (transposed) + exp + PV, per s-block
            po = pout.tile([H + 1, CH], F32, tag="po")
            nsb = (tb + 1) * BPC
            for si in range(nsb):
                lo = max(0, (si - tb * BPC) * P)
                ps = psc.tile([P, CH], F32, tag="sc")
                w = wei.tile([P, CH], BF16, tag="w")
                nc.tensor.matmul(
                    ps[:, lo:CH],
                    lhsT=kT_c[si // BPC][:, (si % BPC) * P : (si % BPC + 1) * P],
                    rhs=qT_c[tb][:, lo:CH],
                    start=True, stop=True,
                )
                nc.scalar.activation(out=w[:, lo:CH], in_=ps[:, lo:CH],
                                     func=EXP, scale=SCALE)
                if si >= tb * BPC:  # diagonal block: zero below-diagonal (t < s)
                    nc.gpsimd.affine_select(
                        out=w[:, lo : lo + P],
                        in_=w[:, lo : lo + P],
                        compare_op=mybir.AluOpType.is_ge,
                        fill=0.0,
                        base=0,
                        # keep where (col - row) >= 0
                        pattern=[[1, P]],
                        channel_multiplier=-1,
                    )
                nc.tensor.matmul(po[:, lo:CH], lhsT=v_c[si // BPC][:, si % BPC, :],
                                 rhs=w[:, lo:CH],
                                 start=(si == 0), stop=(si == nsb - 1))
            # ---- finalize chunk: copy outT+sums to SBUF and store; the cheap
            # per-row divide + transpose happens host-side during unshard.
            os_ = fin.tile([H + 1, CH], F32, tag="ot")
            nc.vector.tensor_copy(out=os_, in_=po)
            nc.sync.dma_start(out=out_d[:, tb * CH : (tb + 1) * CH], in_=os_)
    return nc


_NC_CACHE = []


def _get_nc():
    if not _NC_CACHE:
        nc = build_nc()
        nc.finalize()  # bacc compile: register allocation, DCE
        _NC_CACHE.append(nc)
    return _NC_CACHE[0]


def make_in_maps(inputs):
    x = np.ascontiguousarray(np.asarray(inputs["x"], dtype=np.float32))
    wq = np.asarray(inputs["Wq"], dtype=np.float32)
    wk = np.asarray(inputs["Wk"], dtype=np.float32)
    wv = np.asarray(inputs["Wv"], dtype=np.float32)
    # host-side repack into the on-chip stationary layout [p, jc, h]
    wqk = np.ascontiguousarray(np.concatenate(
        [wq.reshape(NC, P, H).transpose(1, 0, 2), wk.reshape(NC, P, H).transpose(1, 0, 2)],
        axis=2))
    wv2 = np.ascontiguousarray(wv.reshape(NC, P, H).transpose(1, 0, 2))
    return [{"x": np.ascontiguousarray(x[b]), "Wqk": wqk, "Wv2": wv2} for b in range(B)]


def kernel(**inputs):
    from concourse.bass_utils import run_bass_kernel_spmd

    nc = _get_nc()
    res = run_bass_kernel_spmd(nc, make_in_maps(inputs), core_ids=list(range(B)))
    return postprocess([res.results[b]["outT"] for b in range(B)])


def postprocess(outTs):
    outs = []
    for oT in outTs:
        outs.append((oT[0:H, :] / oT[H : H + 1, :]).T.astype(np.float32))
    return np.stack(outs, axis=0)


if __name__ == "__main__":
    import os
    os.makedirs("/tmp/neffdir3", exist_ok=True)
    from concourse.bass_utils import compile_bass_kernel

    nc = _get_nc()
    print("build OK, instructions:",
          sum(len(bb.instructions) for bb in nc.m.functions[0].blocks))
    print("COMPILED:", compile_bass_kernel(nc, "/tmp/neffdir3"))
